# revision 21
# baseline (speedup 1.0000x reference)
"""Trainium2 Bass kernel for spatial multi-head self-attention
(conv1x1 qkv -> 4-head attention over n=4096 tokens -> conv1x1 out + residual).

Sharding: 8 cores = 2 batches x 4 heads; each core runs one (batch, head)
attention and emits the UN-normalized head context [V^T|1]P (33 rows: 32 dims
+ softmax denominator row). Host pre/epilogue: the 1x1 convs (qkv projection
and output projection), normalization, head-sum, bias + residual -- tiny
numpy GEMMs next to the O(n^2) attention the device runs.

v7 = v1's proven attention pipeline with the device prologue removed:
q4/k4 (4x partition-replicated, bf16) and vT1 ([V^T|1], bf16) are built on
host and DMA'd in, so the device runs ONLY the sim/exp/AV pipeline:
  per i-tile (512 tokens): 11 sim groups of <=3 j-tiles, double-buffered
  through two PSUM pools (strict ping-pong); softmax exp is COLUMN-SPLIT
  per group across ACT (native Exp, first ~70%) and DVE (Schraudolph
  tensor_scalar -> int16 bitcast bf16, ~3% rel err) running concurrently;
  sim matmuls are emitted two groups ahead of the exp->AV pair so the
  in-order PE queue never parks AV (which waits on exp) in front of sims.
"""

import numpy as np

B, C, H, W = 2, 128, 64, 64
N = H * W            # 4096
HEADS = 4
DH = 32              # head dim
NT = 512             # i-tile width
NIT = N // NT        # 8 i-tiles
JT = 128             # j-tile width
NJT = N // JT        # 32 j-tiles
GROUPS = [3, 3, 3, 3, 3, 3, 3, 3, 3, 3, 2]    # j-tiles per sim/exp group
SCALE = DH ** -0.5
EXPA = 128.0 / np.log(2.0)   # Schraudolph bf16: bits = rint(s*EXPA + EXPC)
EXPC = 16248.6

_CACHE = {}


def _build():
    if "nc" in _CACHE:
        return _CACHE["nc"]

    import concourse.bacc as bacc
    import concourse.mybir as mybir
    import concourse.tile as tile

    F32 = mybir.dt.float32
    BF16 = mybir.dt.bfloat16
    I16 = mybir.dt.int16
    AF = mybir.ActivationFunctionType
    MULT = mybir.AluOpType.mult
    ADD = mybir.AluOpType.add

    nc = bacc.Bacc("TRN2", target_bir_lowering=False, debug=False, num_devices=8)

    q_in = nc.dram_tensor("q_in", [128, N], BF16, kind="ExternalInput")
    k_in = nc.dram_tensor("k_in", [128, N], BF16, kind="ExternalInput")
    v_in = nc.dram_tensor("v_in", [128, NJT * 33], BF16, kind="ExternalInput")
    o_out = nc.dram_tensor("o_out", [33, N], F32, kind="ExternalOutput")

    with tile.TileContext(nc) as tc:
        with (
            tc.tile_pool(name="const", bufs=1) as cp,
            tc.tile_pool(name="work", bufs=2) as wp,
            tc.tile_pool(name="work3", bufs=3) as wp3,
            tc.tile_pool(name="ps_a", bufs=3, space="PSUM") as psA,
            tc.tile_pool(name="ps_d", bufs=1, space="PSUM") as psD,
            tc.tile_pool(name="ps_o", bufs=1, space="PSUM") as ps_o,
        ):
            # ---- q4/k4/vT1 straight from HBM (host-projected) ----
            k4 = cp.tile([128, N], BF16, tag="k4")
            for ci in range(2):
                s = slice(ci * (N // 2), (ci + 1) * (N // 2))
                nc.sync.dma_start(k4[:, s], k_in.ap()[:, s])
            q4 = cp.tile([128, N], BF16, tag="q4")
            for ci in range(2):
                s = slice(ci * (N // 2), (ci + 1) * (N // 2))
                nc.scalar.dma_start(q4[:, s], q_in.ap()[:, s])
            vT1 = cp.tile([128, NJT, 33], BF16, tag="vT1")
            nc.sync.dma_start(vT1[:], v_in.ap())

            # ---- attention over i-tiles (software-pipelined emission) ----
            descs = []
            for it in range(NIT):
                jbase = 0
                for g, gs in enumerate(GROUPS):
                    descs.append((it, g, jbase, gs))
                    jbase += gs
            s_handles = {}
            o_handles = {}

            def emit_sim(k):
                it, g, jbase, gs = descs[k]
                si = slice(it * NT, (it + 1) * NT)
                s_psA = psA.tile([128, 1024], F32, tag="a", name=f"sa{k}")
                s_psD = (psD.tile([128, 512], F32, tag="d", name=f"sd{k}")
                         if gs == 3 else None)
                for m in range(gs):
                    j = jbase + m
                    dst = (s_psA[:, NT * m:NT * (m + 1)] if m < 2
                           else s_psD[:])
                    nc.tensor.matmul(
                        dst,
                        k4[32 * m:32 * m + 32, j * JT:(j + 1) * JT],
                        q4[32 * m:32 * m + 32, si],
                        start=True, stop=True,
                        tile_position=(32 * m, 0))
                s_handles[k] = (s_psA, s_psD)

            def emit_epilogue(it):
                o_sb = wp.tile([33, NT], F32, tag="o_sb", name=f"ob{it}")
                nc.vector.tensor_copy(o_sb[:], o_handles[it][0:33, :])
                nc.sync.dma_start(
                    o_out.ap()[:, it * NT:(it + 1) * NT], o_sb[:])

            emit_sim(0)
            emit_sim(1)
            for k in range(len(descs)):
                it, g, jbase, gs = descs[k]
                s_psA, s_psD = s_handles.pop(k)
                pT = wp3.tile([128, 2048], BF16, tag="pT", name=f"p{k}")
                nc.scalar.activation(pT[:, 0:1024], s_psA[:], AF.Exp)
                if s_psD is not None:
                    nc.vector.tensor_scalar(
                        pT[:, 1024:1536].bitcast(I16),
                        s_psD[:], EXPA, EXPC, MULT, ADD)
                if g == 0:
                    o_handles[it] = ps_o.tile([128, NT], F32, tag="o",
                                              name=f"o{it}")
                if k + 2 < len(descs):
                    emit_sim(k + 2)
                o_ps = o_handles[it]
                for m in range(gs):
                    j = jbase + m
                    nc.tensor.matmul(
                        o_ps[0:33, :],
                        vT1[:, j, :],
                        pT[:, NT * m:NT * (m + 1)],
                        start=(j == 0), stop=(j == NJT - 1),
                        skip_group_check=True)
                if g == len(GROUPS) - 1:
                    emit_epilogue(it)
            pass

    nc.compile()
    _CACHE["nc"] = nc
    return nc


def make_in_maps(x, w_qkv, w_out, b_out):
    import ml_dtypes
    bf16 = ml_dtypes.bfloat16
    x = np.asarray(x, dtype=np.float32)
    w_qkv = np.asarray(w_qkv, dtype=np.float32)

    xf = x.reshape(B, C, N)
    wq = w_qkv[0:C].reshape(HEADS, DH, C)
    wk = w_qkv[C:2 * C].reshape(HEADS, DH, C)
    wv = w_qkv[2 * C:3 * C].reshape(HEADS, DH, C)

    in_maps = []
    for core in range(8):
        b_i, h_i = divmod(core, HEADS)
        xb = xf[b_i]
        q = (wq[h_i] * SCALE) @ xb          # [32, N]
        k = wk[h_i] @ xb
        v = wv[h_i] @ xb
        vt = v.reshape(DH, NJT, JT).transpose(2, 1, 0)   # [128, NJT, 32]
        vT1 = np.concatenate(
            [vt, np.ones((JT, NJT, 1), np.float32)], axis=2)
        in_maps.append({
            "q_in": np.ascontiguousarray(np.tile(q, (4, 1))).astype(bf16),
            "k_in": np.ascontiguousarray(np.tile(k, (4, 1))).astype(bf16),
            "v_in": np.ascontiguousarray(
                vT1.reshape(JT, NJT * 33)).astype(bf16),
        })
    return in_maps


def kernel(x, w_qkv, w_out, b_out):
    from concourse.bass_utils import run_bass_kernel_spmd

    x = np.asarray(x, dtype=np.float32)
    w_out = np.asarray(w_out, dtype=np.float32)
    b_out = np.asarray(b_out, dtype=np.float32)
    xf = np.ascontiguousarray(x.reshape(B, C, N))

    in_maps = make_in_maps(x, w_qkv, w_out, b_out)

    nc = _build()
    res = run_bass_kernel_spmd(nc, in_maps, core_ids=list(range(8)))

    # host epilogue: normalize, output-project, sum heads, bias + residual
    outf = np.tile(b_out[None, :, None], (B, 1, N)) + xf
    for core in range(8):
        b_i, h_i = divmod(core, HEADS)
        o33 = res.results[core]["o_out"]
        attn = o33[0:DH] / o33[DH][None, :]            # normalize
        woh = w_out[:, h_i * DH:(h_i + 1) * DH]        # [C, DH]
        outf[b_i] += woh @ attn
    return outf.reshape(B, C, H, W).astype(np.float32)


# revision 22
# speedup vs baseline: 1.2631x; 1.2631x over previous
"""Trainium2 Bass kernel for spatial multi-head self-attention
(conv1x1 qkv -> 4-head attention over n=4096 tokens -> conv1x1 out + residual).

Sharding: 8 cores = 2 batches x 4 heads; each core runs one (batch, head)
attention and emits the UN-normalized head context [V^T|1]P (33 rows: 32 dims
+ softmax denominator row). Host pre/epilogue: the 1x1 convs (qkv projection
and output projection), normalization, head-sum, bias + residual -- tiny
numpy GEMMs next to the O(n^2) attention the device runs.

v7 = v1's proven attention pipeline with the device prologue removed:
q4/k4 (4x partition-replicated, bf16) and vT1 ([V^T|1], bf16) are built on
host and DMA'd in, so the device runs ONLY the sim/exp/AV pipeline:
  per i-tile (512 tokens): 11 sim groups of <=3 j-tiles, double-buffered
  through two PSUM pools (strict ping-pong); softmax exp is COLUMN-SPLIT
  per group across ACT (native Exp, first ~70%) and DVE (Schraudolph
  tensor_scalar -> int16 bitcast bf16, ~3% rel err) running concurrently;
  sim matmuls are emitted two groups ahead of the exp->AV pair so the
  in-order PE queue never parks AV (which waits on exp) in front of sims.
"""

import numpy as np

B, C, H, W = 2, 128, 64, 64
N = H * W            # 4096
HEADS = 4
DH = 32              # head dim
NT = 512             # i-tile width
NIT = N // NT        # 8 i-tiles
JT = 128             # j-tile width
NJT = N // JT        # 32 j-tiles
GROUPS = [3, 3, 3, 3, 3, 3, 3, 3, 3, 3, 2]    # j-tiles per sim/exp group
SCALE = DH ** -0.5
EXPA = 128.0 / np.log(2.0)   # Schraudolph bf16: bits = rint(s*EXPA + EXPC)
EXPC = 16248.6

_CACHE = {}


def _build():
    if "nc" in _CACHE:
        return _CACHE["nc"]

    import concourse.bacc as bacc
    import concourse.mybir as mybir
    import concourse.tile as tile

    F32 = mybir.dt.float32
    BF16 = mybir.dt.bfloat16
    I16 = mybir.dt.int16
    AF = mybir.ActivationFunctionType
    MULT = mybir.AluOpType.mult
    ADD = mybir.AluOpType.add

    nc = bacc.Bacc("TRN2", target_bir_lowering=False, debug=False, num_devices=8)

    q_in = nc.dram_tensor("q_in", [128, N], BF16, kind="ExternalInput")
    k_in = nc.dram_tensor("k_in", [128, N], BF16, kind="ExternalInput")
    v_in = nc.dram_tensor("v_in", [128, NJT * 33], BF16, kind="ExternalInput")
    o_out = nc.dram_tensor("o_out", [33, N], F32, kind="ExternalOutput")

    with tile.TileContext(nc) as tc:
        with (
            tc.tile_pool(name="const", bufs=1) as cp,
            tc.tile_pool(name="work", bufs=2) as wp,
            tc.tile_pool(name="work3", bufs=3) as wp3,
            tc.tile_pool(name="ps_a", bufs=3, space="PSUM") as psA,
            tc.tile_pool(name="ps_d", bufs=1, space="PSUM") as psD,
            tc.tile_pool(name="ps_o", bufs=1, space="PSUM") as ps_o,
        ):
            # ---- warm the ACT Exp table while DMAs run ----
            warm = cp.tile([128, 2], F32, tag="warm")
            nc.gpsimd.memset(warm[:], 0.0)
            warm_o = cp.tile([128, 2], BF16, tag="warm_o")
            nc.scalar.activation(warm_o[:], warm[:], AF.Exp)

            # ---- q4/k4/vT1 straight from HBM (host-projected) ----
            # first chunks sized to unblock the first sim groups quickly
            k4 = cp.tile([128, N], BF16, tag="k4")
            for s in (slice(0, 1024), slice(1024, 2048), slice(2048, N)):
                nc.sync.dma_start(k4[:, s], k_in.ap()[:, s])
            q4 = cp.tile([128, N], BF16, tag="q4")
            for s in (slice(0, 512), slice(512, 2048), slice(2048, N)):
                nc.scalar.dma_start(q4[:, s], q_in.ap()[:, s])
            vT1 = cp.tile([128, NJT, 33], BF16, tag="vT1")
            nc.sync.dma_start(vT1[:], v_in.ap())

            # ---- attention over i-tiles (software-pipelined emission) ----
            descs = []
            for it in range(NIT):
                jbase = 0
                for g, gs in enumerate(GROUPS):
                    descs.append((it, g, jbase, gs))
                    jbase += gs
            s_handles = {}
            o_handles = {}

            def emit_sim(k):
                it, g, jbase, gs = descs[k]
                si = slice(it * NT, (it + 1) * NT)
                s_psA = psA.tile([128, 1024], F32, tag="a", name=f"sa{k}")
                s_psD = (psD.tile([128, 512], F32, tag="d", name=f"sd{k}")
                         if gs == 3 else None)
                for m in range(gs):
                    j = jbase + m
                    dst = (s_psA[:, NT * m:NT * (m + 1)] if m < 2
                           else s_psD[:])
                    nc.tensor.matmul(
                        dst,
                        k4[32 * m:32 * m + 32, j * JT:(j + 1) * JT],
                        q4[32 * m:32 * m + 32, si],
                        start=True, stop=True,
                        tile_position=(32 * m, 0))
                s_handles[k] = (s_psA, s_psD)

            def emit_epilogue(it):
                o_sb = wp.tile([33, NT], F32, tag="o_sb", name=f"ob{it}")
                nc.scalar.copy(o_sb[:], o_handles[it][0:33, :])
                nc.sync.dma_start(
                    o_out.ap()[:, it * NT:(it + 1) * NT], o_sb[:])

            emit_sim(0)
            emit_sim(1)
            for k in range(len(descs)):
                it, g, jbase, gs = descs[k]
                s_psA, s_psD = s_handles.pop(k)
                pT = wp3.tile([128, 2048], BF16, tag="pT", name=f"p{k}")
                nc.scalar.activation(pT[:, 0:1024], s_psA[:], AF.Exp)
                if s_psD is not None:
                    nc.vector.tensor_scalar(
                        pT[:, 1024:1536].bitcast(I16),
                        s_psD[:], EXPA, EXPC, MULT, ADD)
                if g == 0:
                    if it > 0:
                        emit_epilogue(it - 1)
                    o_handles[it] = ps_o.tile([128, NT], F32, tag="o",
                                              name=f"o{it}")
                if k + 2 < len(descs):
                    emit_sim(k + 2)
                o_ps = o_handles[it]
                for m in range(gs):
                    j = jbase + m
                    nc.tensor.matmul(
                        o_ps[0:33, :],
                        vT1[:, j, :],
                        pT[:, NT * m:NT * (m + 1)],
                        start=(j == 0), stop=(j == NJT - 1),
                        skip_group_check=True)
            emit_epilogue(NIT - 1)

    nc.compile()
    _CACHE["nc"] = nc
    return nc


def make_in_maps(x, w_qkv, w_out, b_out):
    import ml_dtypes
    bf16 = ml_dtypes.bfloat16
    x = np.asarray(x, dtype=np.float32)
    w_qkv = np.asarray(w_qkv, dtype=np.float32)

    xf = x.reshape(B, C, N)
    wq = w_qkv[0:C].reshape(HEADS, DH, C)
    wk = w_qkv[C:2 * C].reshape(HEADS, DH, C)
    wv = w_qkv[2 * C:3 * C].reshape(HEADS, DH, C)

    in_maps = []
    for core in range(8):
        b_i, h_i = divmod(core, HEADS)
        xb = xf[b_i]
        q = (wq[h_i] * SCALE) @ xb          # [32, N]
        k = wk[h_i] @ xb
        v = wv[h_i] @ xb
        vt = v.reshape(DH, NJT, JT).transpose(2, 1, 0)   # [128, NJT, 32]
        vT1 = np.concatenate(
            [vt, np.ones((JT, NJT, 1), np.float32)], axis=2)
        in_maps.append({
            "q_in": np.ascontiguousarray(np.tile(q, (4, 1))).astype(bf16),
            "k_in": np.ascontiguousarray(np.tile(k, (4, 1))).astype(bf16),
            "v_in": np.ascontiguousarray(
                vT1.reshape(JT, NJT * 33)).astype(bf16),
        })
    return in_maps


def kernel(x, w_qkv, w_out, b_out):
    from concourse.bass_utils import run_bass_kernel_spmd

    x = np.asarray(x, dtype=np.float32)
    w_out = np.asarray(w_out, dtype=np.float32)
    b_out = np.asarray(b_out, dtype=np.float32)
    xf = np.ascontiguousarray(x.reshape(B, C, N))

    in_maps = make_in_maps(x, w_qkv, w_out, b_out)

    nc = _build()
    res = run_bass_kernel_spmd(nc, in_maps, core_ids=list(range(8)))

    # host epilogue: normalize, output-project, sum heads, bias + residual
    outf = np.tile(b_out[None, :, None], (B, 1, N)) + xf
    for core in range(8):
        b_i, h_i = divmod(core, HEADS)
        o33 = res.results[core]["o_out"]
        attn = o33[0:DH] / o33[DH][None, :]            # normalize
        woh = w_out[:, h_i * DH:(h_i + 1) * DH]        # [C, DH]
        outf[b_i] += woh @ attn
    return outf.reshape(B, C, H, W).astype(np.float32)
